# revision 12
# baseline (speedup 1.0000x reference)
"""Trainium2 Bass kernel for nn_Decoder_36636071035490.

Reference computes, for workers i and task/edge (j,l):
    z = worker_feature @ W            # [2000, 1]
    p1 = sigmoid(z + b)
    p2 = (1 - p1) / 9
    P[i, j, l] = p1_i^tau_jl * p2_i^(1 - tau_jl)      # [2000, 5000, 10] f32

Identities used on device (exact in exact arithmetic):
    a_i  = (z_i + b) + ln 9           # ln(p1/p2), since logit(sigmoid(x)) = x
    p2_i = 1 / (9 * (1 + exp(z_i + b)))
    c_i  = ln(p2_i) = -ln(1 + exp(z_i + b)) - ln 9
    P[i, f] = exp(a_i * tau_f + c_i)  = p2_i * exp(a_i * tau_f)

Sharding: by output columns (task*edge flattened, 50000 -> 8 x 6250); every
core computes the cheap per-worker scalars for all 2000 workers (replicated
matvec) and produces the full-height [2000, 6250] slab of P.

Device-side schedule (tuned against SDMA-engine traces):
- workers are processed in blocks interleaved two-per-partition (partition
  p of a block holds workers off+2p and off+2p+1), so a block's store is
  one DMA whose per-partition descriptor is a CONTIGUOUS 50 KB range of the
  output (near the 64 KB cap).  128-partition stores get the port-aligned
  16-engine split and run 26.7 GB/s per SDMA engine; any other partition
  count falls back to a consecutive split at roughly HALF that rate on
  (largest divisor <= 16) engines - measured.
- SDMA engine 15 also serves the dynamic-queue rings and only manages
  ~21 GB/s, so it must carry fewer bytes than engines 0-14.  The only
  full-rate stores are 128-partition ones (which load all 16 engines
  evenly), so the skew comes from ONE half-rate block: 7 blocks x 256
  workers (128 partitions, all 16 engines) + 1 block x 208 workers (104
  partitions -> engines 0..12 only, engine 15 idle).  Exact cover of 2000
  workers, no double stores.  Engine 15 ends at ~2.8 MB vs ~3.2+ MB for
  engines 0-12, matching the measured speed gap.
- tau columns [0:2048] come from a stride-0 broadcast DMA; the rest are
  broadcast by the idle PE (ones[1,128]^T @ tau chunk -> PSUM bank, DVE
  copies PSUM->SBUF).  fp32 PE matmuls are 2-pass, so this chain takes
  ~17 us - the DMA part covers the early ACTs while it runs.
- block 0 ramps with scale-only Exp quarters multiplied by p2 on the DVE
  (no dependence on the Ln table); each quarter is stored immediately.
  The Ln chain producing the bias c runs between block-0 quarters where
  ScalarE would idle anyway.  Blocks 1+ run pure exp(a*tau+c) ACTs with no
  steady-state DVE traffic (a per-block DVE post-multiply measurably
  degrades concurrent SDMA store throughput ~25%).
- the tiny prologue (matvec z, a, p2) is emitted BEFORE the PSUM copy
  chain so the tile scheduler runs it the moment the worker features land
  instead of behind 13 PSUM copies.
"""

import numpy as np

WORKERS = 2000
TASKS = 5000
ET = 10
AB = 64
NCORES = 8
F = TASKS * ET  # 50000 output cols
FS = F // NCORES  # 6250 cols per core
LN9 = float(np.log(9.0))

# 7 blocks of 256 workers (128 partitions, full-rate stores) + 1 block of
# 208 workers (104 partitions -> engines 0..12, skews work off engine 15)
BLOCKS = [(g * 256, 128) for g in range(7)] + [(1792, 104)]
# processing order: ramp block, one more 128p block, then the slow block
# early (its packets sit in the per-engine FIFOs; order doesn't change
# engine finish times but keeps the kernel tail all full-rate)
ORDER = [0, 1, 7, 2, 3, 4, 5, 6]
TB = 3126  # tau columns broadcast by DMA; the rest go through the PE (fp16)
_Q = [0, 1563, 3126, 4688, FS]  # block-0 ramp quarters
_H = [0, FS // 2, FS]  # block-1 halves

_CACHE = {}


def _build_nc():
    import concourse.bass as bass
    import concourse.mybir as mybir
    from concourse import bacc
    from concourse.tile import TileContext
    from contextlib import ExitStack

    f32 = mybir.dt.float32
    AF = mybir.ActivationFunctionType
    OP = mybir.AluOpType

    nc = bacc.Bacc("TRN2")
    wk = nc.dram_tensor("wk", [WORKERS, AB], f32, kind="ExternalInput")
    f16 = mybir.dt.float16
    tf0 = nc.dram_tensor("tf0", [1, FS], f32, kind="ExternalInput")
    tf16 = nc.dram_tensor("tf16", [1, FS - TB], f16, kind="ExternalInput")
    onesd = nc.dram_tensor("ones", [1, 128], f16, kind="ExternalInput")
    Wd = nc.dram_tensor("W", [AB, 1], f32, kind="ExternalInput")
    bd = nc.dram_tensor("b", [1], f32, kind="ExternalInput")
    out = nc.dram_tensor("out", [WORKERS, FS], f32, kind="ExternalOutput")

    with TileContext(nc) as tc, ExitStack() as ctx:
        const = ctx.enter_context(tc.tile_pool(name="const", bufs=1))
        psum = ctx.enter_context(
            tc.tile_pool(name="ps", bufs=4, space=bass.MemorySpace.PSUM)
        )
        stage_p = ctx.enter_context(tc.tile_pool(name="stagep", bufs=3))

        # ---- input DMAs.  Order matters: the sync queue is FIFO and the
        # PE chain + prologue are the ramp critical path, so the tiny loads
        # they need go first, then the worker features, then the 1 MB tau
        # broadcast, which only gates the very first ACT quarter.
        taub = const.tile([128, FS], f32, name="taub")
        t16 = const.tile([1, FS - TB], f16, name="t16")
        ones_t = const.tile([1, 128], f16, name="ones")
        wka = const.tile([128, 8, 2 * AB], f32, name="wka")
        Wb = const.tile([128, AB], f32, name="Wb")
        bcol = const.tile([128, 1], f32, name="bcol")

        nc.sync.dma_start(out=t16, in_=tf16[:])
        nc.sync.dma_start(out=ones_t, in_=onesd[:])
        nc.sync.dma_start(out=bcol, in_=bd[:].to_broadcast((128, 1)))
        nc.sync.dma_start(
            out=Wb, in_=Wd[:].rearrange("a b -> b a").to_broadcast((128, AB))
        )
        # worker features, two workers per (partition, block) as 512B
        # descriptors: wka[p, g, (c a)] = wk[off_g + 2p + c, a]
        nc.sync.dma_start(
            out=wka[:, 0:7, :],
            in_=wk[0:1792, :].rearrange("(g p c) a -> p g (c a)", p=128, c=2),
        )
        nc.sync.dma_start(
            out=wka[0:104, 7:8, :],
            in_=wk[1792:2000, :].rearrange("(g p c) a -> p g (c a)", p=104, c=2),
        )
        # pad unused lanes of the 104-partition block with valid data (any
        # rows) so the prologue never reads uninitialized SBUF
        nc.sync.dma_start(
            out=wka[104:128, 7:8, :],
            in_=wk[0:48, :].rearrange("(g p c) a -> p g (c a)", p=24, c=2),
        )
        nc.sync.dma_start(
            out=taub[:, 0:TB], in_=tf0[0:1, 0:TB].to_broadcast((128, TB))
        )

        # ---- per-worker scalars (emitted before the PSUM copies so the
        # scheduler runs them as soon as wka lands).  Column j = 2g + c_par
        # holds worker off_g + 2p + c_par.
        NJ = 16
        wk16 = wka[:].rearrange("p g (c a) -> p (g c) a", a=AB)
        WbT = bass.AP(
            tensor=Wb.tensor,
            offset=Wb.offset,
            ap=[list(Wb.ap[0]), [0, NJ], [1, AB]],
        )
        prod = const.tile([128, NJ, AB], f32, name="prod")
        nc.vector.tensor_mul(prod, wk16, WbT)
        zb_ = const.tile([128, NJ], f32, name="zb")
        nc.vector.reduce_sum(
            out=zb_.rearrange("p (t o) -> p t o", o=1),
            in_=prod,
            axis=mybir.AxisListType.X,
        )
        a_ = const.tile([128, NJ], f32, name="a")
        nc.vector.tensor_scalar(
            out=a_, in0=zb_, scalar1=bcol, scalar2=LN9, op0=OP.add, op1=OP.add
        )
        eb_ = const.tile([128, NJ], f32, name="eb")
        nc.scalar.activation(out=eb_, in_=zb_, func=AF.Exp, bias=bcol, scale=1.0)
        den_ = const.tile([128, NJ], f32, name="den")
        nc.vector.tensor_scalar(
            out=den_, in0=eb_, scalar1=1.0, scalar2=9.0, op0=OP.add, op1=OP.mult
        )
        p2_ = const.tile([128, NJ], f32, name="p2")
        nc.vector.reciprocal(out=p2_, in_=den_)

        # ---- PE partition-broadcast of tau[TB:FS]
        CH = 512  # one PSUM bank of f32
        chunks = []
        for n0 in range(TB, FS, CH):
            n1 = min(n0 + CH, FS)
            ps = psum.tile([128, CH], f32, name="ps", tag="ps")
            nc.tensor.matmul(
                ps[:, 0 : n1 - n0],
                ones_t[:],
                t16[0:1, n0 - TB : n1 - TB],
                start=True,
                stop=True,
            )
            chunks.append((n0, n1, ps))
        for n0, n1, ps in chunks:
            nc.vector.tensor_copy(taub[:, n0:n1], ps[:, 0 : n1 - n0])

        # ---- block 0 ramp: scale-only Exp quarters, DVE multiplies by p2.
        # The Ln chain producing c_ is emitted between quarters, where
        # ScalarE stalls on the PE broadcast chain anyway.
        stg0 = stage_p.tile([128, 2, FS], f32, name="stg0", tag="stg")
        dst0 = out[0:256, :].rearrange("(p c) f -> p c f", c=2)
        lb_ = const.tile([128, NJ], f32, name="lb")
        c_ = const.tile([128, NJ], f32, name="c")

        def _b0_quarter(qi):
            c0, c1 = _Q[qi], _Q[qi + 1]
            for cpar in (0, 1):
                nc.scalar.activation(
                    out=stg0[:, cpar, c0:c1],
                    in_=taub[:, c0:c1],
                    func=AF.Exp,
                    scale=a_[:, cpar : cpar + 1],
                )
                nc.vector.tensor_scalar_mul(
                    stg0[:, cpar, c0:c1],
                    stg0[:, cpar, c0:c1],
                    p2_[:, cpar : cpar + 1],
                )
                nc.sync.dma_start(
                    out=dst0[:, cpar, c0:c1], in_=stg0[:, cpar, c0:c1]
                )

        _b0_quarter(0)
        _b0_quarter(1)
        # ln(1 + e^(z+b)) via Ln(eb_ + 1); c = -ln(..) - ln 9
        nc.scalar.activation(out=lb_, in_=eb_, func=AF.Ln, bias=1.0, scale=1.0)
        nc.vector.tensor_scalar(
            out=c_, in0=lb_, scalar1=-1.0, scalar2=-LN9, op0=OP.mult, op1=OP.add
        )
        _b0_quarter(2)
        _b0_quarter(3)

        # ---- remaining blocks: pure exp(a*tau + c) ACTs, no DVE traffic.
        for g in ORDER[1:]:
            off, P = BLOCKS[g]
            stg = stage_p.tile([128, 2, FS], f32, name="stg", tag="stg")
            dst = out[off : off + 2 * P, :].rearrange("(p c) f -> p c f", c=2)
            sp = _H if g == 1 else [0, FS]
            for cpar in (0, 1):
                j = 2 * g + cpar
                for c0, c1 in zip(sp[:-1], sp[1:]):
                    nc.scalar.activation(
                        out=stg[0:P, cpar, c0:c1],
                        in_=taub[0:P, c0:c1],
                        func=AF.Exp,
                        bias=c_[0:P, j : j + 1],
                        scale=a_[0:P, j : j + 1],
                    )
                    if len(sp) > 2:
                        nc.sync.dma_start(
                            out=dst[:, cpar, c0:c1], in_=stg[0:P, cpar, c0:c1]
                        )
            if len(sp) == 2:
                nc.sync.dma_start(out=dst, in_=stg[0:P, :, :])

    nc.compile()
    return nc


def _get_nc():
    if "nc" not in _CACHE:
        _CACHE["nc"] = _build_nc()
    return _CACHE["nc"]


def _make_in_maps(inputs_arr, W, b):
    wk = np.ascontiguousarray(inputs_arr[:WORKERS, :AB], dtype=np.float32)
    tau_flat = np.ascontiguousarray(
        inputs_arr[WORKERS:, :ET], dtype=np.float32
    ).reshape(F)
    W = np.ascontiguousarray(W, dtype=np.float32)
    b = np.ascontiguousarray(b, dtype=np.float32)
    ones = np.ones((1, 128), dtype=np.float16)
    maps = []
    for c in range(NCORES):
        tf0 = np.ascontiguousarray(tau_flat[c * FS : (c + 1) * FS]).reshape(1, FS)
        tf16 = np.ascontiguousarray(tf0[:, TB:].astype(np.float16))
        maps.append(
            {"wk": wk, "tf0": tf0, "tf16": tf16, "ones": ones, "W": W, "b": b}
        )
    return maps


def _run(inputs_arr, W, b, **kwargs):
    from concourse import bass_utils

    nc = _get_nc()
    in_maps = _make_in_maps(inputs_arr, W, b)
    return bass_utils.run_bass_kernel_spmd(
        nc, in_maps, core_ids=list(range(NCORES)), **kwargs
    )


def kernel(inputs, W, b):
    inputs_arr = np.asarray(inputs, dtype=np.float32)
    last_err = None
    for _ in range(3):  # retry transient device failures
        try:
            res = _run(inputs_arr, np.asarray(W), np.asarray(b))
            break
        except Exception as e:  # noqa: BLE001
            last_err = e
    else:
        raise last_err
    out = np.concatenate([r["out"] for r in res.results], axis=1)
    return out.reshape(WORKERS, TASKS, ET)


# revision 13
# speedup vs baseline: 1.0288x; 1.0288x over previous
"""Trainium2 Bass kernel for nn_Decoder_36636071035490.

Reference computes, for workers i and task/edge (j,l):
    z = worker_feature @ W            # [2000, 1]
    p1 = sigmoid(z + b)
    p2 = (1 - p1) / 9
    P[i, j, l] = p1_i^tau_jl * p2_i^(1 - tau_jl)      # [2000, 5000, 10] f32

Identities used on device (exact in exact arithmetic):
    a_i  = (z_i + b) + ln 9           # ln(p1/p2), since logit(sigmoid(x)) = x
    p2_i = 1 / (9 * (1 + exp(z_i + b)))
    c_i  = ln(p2_i) = -ln(1 + exp(z_i + b)) - ln 9
    P[i, f] = exp(a_i * tau_f + c_i)  = p2_i * exp(a_i * tau_f)

Sharding: by output columns (task*edge flattened, 50000 -> 8 x 6250); every
core computes the cheap per-worker scalars for all 2000 workers (replicated
matvec) and produces the full-height [2000, 6250] slab of P.

Device-side schedule (tuned against SDMA-engine traces):
- workers are processed in blocks interleaved two-per-partition (partition
  p of a block holds workers off+2p and off+2p+1), so a block's store is
  one DMA whose per-partition descriptor is a CONTIGUOUS 50 KB range of the
  output (near the 64 KB cap).  128-partition stores get the port-aligned
  16-engine split and run ~26.7 GB/s per SDMA engine; any other partition
  count falls back to a consecutive split at roughly HALF that rate on
  (largest divisor <= 16) engines - measured.
- SDMA engine 15 also serves the dynamic-queue rings and only manages
  ~21 GB/s, so it must carry fewer bytes than engines 0-14.  The only
  full-rate stores are 128-partition ones (which load all 16 engines
  evenly), so the skew comes from ONE half-rate block: 7 blocks x 256
  workers (128 partitions, all 16 engines) + 1 block x 208 workers (104
  partitions -> engines 0..12 only, engine 15 idle).  Exact cover of 2000
  workers, no double stores.
- tau is passed as fp16 [1, 6250] (worst-case P error ~3e-3, budget 2e-2)
  and broadcast across partitions entirely by the idle PE: single-pass
  fp16 matmuls ones[1,128]^T @ tau-chunk -> PSUM bank, DVE copies out.
  No replicated HBM read at all; the 13-chunk chain finishes by ~17 us.
- the per-worker prologue is SPLIT: a mini-prologue for block 0 only
  (workers 0..255, loaded by a dedicated first-in-queue DMA) unblocks the
  first ACT before the PSUM-copy chain occupies the DVE; the rest of the
  prologue runs later.  Block 0 ramps with scale-only Exp quarters
  multiplied by p2 on the DVE, each quarter stored immediately.  The Ln
  chain producing the bias c runs after the quarters; blocks 1+ run pure
  exp(a*tau+c) ACTs with NO steady-state DVE traffic (a per-block DVE
  post-multiply measurably degrades concurrent SDMA store throughput).
"""

import numpy as np

WORKERS = 2000
TASKS = 5000
ET = 10
AB = 64
NCORES = 8
F = TASKS * ET  # 50000 output cols
FS = F // NCORES  # 6250 cols per core
LN9 = float(np.log(9.0))

# 7 blocks of 256 workers (128 partitions, full-rate stores) + 1 block of
# 208 workers (104 partitions -> engines 0..12, skews work off engine 15)
BLOCKS = [(g * 256, 128) for g in range(7)] + [(1792, 104)]
# processing order: ramp block, one more 128p block, then the slow block
# early (its packets sit in the per-engine FIFOs; order doesn't change
# engine finish times but keeps the kernel tail all full-rate)
ORDER = [0, 1, 7, 2, 3, 4, 5, 6]
_Q = [0, 1563, 3126, 4688, FS]  # block-0 ramp quarters
_H = [0, FS // 2, FS]  # block-1 halves

_CACHE = {}


def _build_nc():
    import concourse.bass as bass
    import concourse.mybir as mybir
    from concourse import bacc
    from concourse.tile import TileContext
    from contextlib import ExitStack

    f32 = mybir.dt.float32
    f16 = mybir.dt.float16
    AF = mybir.ActivationFunctionType
    OP = mybir.AluOpType

    nc = bacc.Bacc("TRN2")
    wk = nc.dram_tensor("wk", [WORKERS, AB], f32, kind="ExternalInput")
    tf16 = nc.dram_tensor("tf16", [1, FS], f16, kind="ExternalInput")
    onesd = nc.dram_tensor("ones", [1, 128], f16, kind="ExternalInput")
    Wd = nc.dram_tensor("W", [AB, 1], f32, kind="ExternalInput")
    bd = nc.dram_tensor("b", [1], f32, kind="ExternalInput")
    out = nc.dram_tensor("out", [WORKERS, FS], f32, kind="ExternalOutput")

    with TileContext(nc) as tc, ExitStack() as ctx:
        const = ctx.enter_context(tc.tile_pool(name="const", bufs=1))
        psum = ctx.enter_context(
            tc.tile_pool(name="ps", bufs=4, space=bass.MemorySpace.PSUM)
        )
        stage_p = ctx.enter_context(tc.tile_pool(name="stagep", bufs=3))

        taub = const.tile([128, FS], f32, name="taub")
        t16 = const.tile([1, FS], f16, name="t16")
        ones_t = const.tile([1, 128], f16, name="ones")
        wka = const.tile([128, 8, 2 * AB], f32, name="wka")
        Wb = const.tile([128, AB], f32, name="Wb")
        bcol = const.tile([128, 1], f32, name="bcol")

        # ---- input DMAs (sync queue is FIFO).  Block-0 workers and the
        # fp16 tau go first: they gate the mini-prologue and the PE chain.
        nc.sync.dma_start(
            out=wka[:, 0:1, :],
            in_=wk[0:256, :].rearrange("(g p c) a -> p g (c a)", p=128, c=2),
        )
        nc.sync.dma_start(out=t16, in_=tf16[:])
        nc.sync.dma_start(out=ones_t, in_=onesd[:])
        nc.sync.dma_start(out=bcol, in_=bd[:].to_broadcast((128, 1)))
        nc.sync.dma_start(
            out=Wb, in_=Wd[:].rearrange("a b -> b a").to_broadcast((128, AB))
        )
        # remaining worker features: wka[p, g, (c a)] = wk[off_g + 2p + c, a]
        nc.sync.dma_start(
            out=wka[:, 1:7, :],
            in_=wk[256:1792, :].rearrange("(g p c) a -> p g (c a)", p=128, c=2),
        )
        nc.sync.dma_start(
            out=wka[0:104, 7:8, :],
            in_=wk[1792:2000, :].rearrange("(g p c) a -> p g (c a)", p=104, c=2),
        )
        # pad unused lanes of the 104-partition block with valid data (any
        # rows) so the prologue never reads uninitialized SBUF
        nc.sync.dma_start(
            out=wka[104:128, 7:8, :],
            in_=wk[0:48, :].rearrange("(g p c) a -> p g (c a)", p=24, c=2),
        )

        # ---- mini-prologue: a and p2 for block 0 only (columns j=0,1).
        NJ = 16
        wk16 = wka[:].rearrange("p g (c a) -> p (g c) a", a=AB)
        WbT2 = bass.AP(
            tensor=Wb.tensor,
            offset=Wb.offset,
            ap=[list(Wb.ap[0]), [0, 2], [1, AB]],
        )
        prod = const.tile([128, NJ, AB], f32, name="prod")
        zb_ = const.tile([128, NJ], f32, name="zb")
        a_ = const.tile([128, NJ], f32, name="a")
        eb_ = const.tile([128, NJ], f32, name="eb")
        nc.vector.tensor_mul(prod[:, 0:2, :], wk16[:, 0:2, :], WbT2)
        nc.vector.reduce_sum(
            out=zb_[:, 0:2].rearrange("p (t o) -> p t o", o=1),
            in_=prod[:, 0:2, :],
            axis=mybir.AxisListType.X,
        )
        nc.vector.tensor_scalar(
            out=a_[:, 0:2],
            in0=zb_[:, 0:2],
            scalar1=bcol,
            scalar2=LN9,
            op0=OP.add,
            op1=OP.add,
        )
        nc.scalar.activation(
            out=eb_[:, 0:2], in_=zb_[:, 0:2], func=AF.Exp, bias=bcol, scale=1.0
        )
        den_ = const.tile([128, 2], f32, name="den")
        nc.vector.tensor_scalar(
            out=den_, in0=eb_[:, 0:2], scalar1=1.0, scalar2=9.0, op0=OP.add, op1=OP.mult
        )
        p2_ = const.tile([128, 2], f32, name="p2")
        nc.vector.reciprocal(out=p2_, in_=den_)

        # ---- PE partition-broadcast of tau (fp16 single-pass matmuls)
        CH = 512  # one PSUM bank of f32
        chunks = []
        for n0 in range(0, FS, CH):
            n1 = min(n0 + CH, FS)
            ps = psum.tile([128, CH], f32, name="ps", tag="ps")
            nc.tensor.matmul(
                ps[:, 0 : n1 - n0],
                ones_t[:],
                t16[0:1, n0:n1],
                start=True,
                stop=True,
            )
            chunks.append((n0, n1, ps))
        for n0, n1, ps in chunks:
            nc.vector.tensor_copy(taub[:, n0:n1], ps[:, 0 : n1 - n0])

        # ---- main prologue: z, a for all 16 columns (j = 2g + c_par holds
        # worker off_g + 2p + c_par)
        WbT = bass.AP(
            tensor=Wb.tensor,
            offset=Wb.offset,
            ap=[list(Wb.ap[0]), [0, NJ - 2], [1, AB]],
        )
        nc.vector.tensor_mul(prod[:, 2:NJ, :], wk16[:, 2:NJ, :], WbT)
        nc.vector.reduce_sum(
            out=zb_[:, 2:NJ].rearrange("p (t o) -> p t o", o=1),
            in_=prod[:, 2:NJ, :],
            axis=mybir.AxisListType.X,
        )
        nc.vector.tensor_scalar(
            out=a_[:, 2:NJ],
            in0=zb_[:, 2:NJ],
            scalar1=bcol,
            scalar2=LN9,
            op0=OP.add,
            op1=OP.add,
        )

        # ---- block 0 ramp: scale-only Exp quarters, DVE multiplies by p2
        stg0 = stage_p.tile([128, 2, FS], f32, name="stg0", tag="stg")
        dst0 = out[0:256, :].rearrange("(p c) f -> p c f", c=2)
        for qi in range(4):
            c0, c1 = _Q[qi], _Q[qi + 1]
            for cpar in (0, 1):
                nc.scalar.activation(
                    out=stg0[:, cpar, c0:c1],
                    in_=taub[:, c0:c1],
                    func=AF.Exp,
                    scale=a_[:, cpar : cpar + 1],
                )
                nc.vector.tensor_scalar_mul(
                    stg0[:, cpar, c0:c1],
                    stg0[:, cpar, c0:c1],
                    p2_[:, cpar : cpar + 1],
                )
                nc.sync.dma_start(
                    out=dst0[:, cpar, c0:c1], in_=stg0[:, cpar, c0:c1]
                )

        # ---- the bias c for blocks 1+: c = -ln(1 + e^(z+b)) - ln 9
        lb_ = const.tile([128, NJ], f32, name="lb")
        c_ = const.tile([128, NJ], f32, name="c")
        nc.scalar.activation(
            out=eb_[:, 2:NJ], in_=zb_[:, 2:NJ], func=AF.Exp, bias=bcol, scale=1.0
        )
        nc.scalar.activation(out=lb_, in_=eb_, func=AF.Ln, bias=1.0, scale=1.0)
        nc.vector.tensor_scalar(
            out=c_, in0=lb_, scalar1=-1.0, scalar2=-LN9, op0=OP.mult, op1=OP.add
        )

        # ---- remaining blocks: pure exp(a*tau + c) ACTs, no DVE traffic
        for g in ORDER[1:]:
            off, P = BLOCKS[g]
            stg = stage_p.tile([128, 2, FS], f32, name="stg", tag="stg")
            dst = out[off : off + 2 * P, :].rearrange("(p c) f -> p c f", c=2)
            sp = _H if g == 1 else [0, FS]
            for cpar in (0, 1):
                j = 2 * g + cpar
                for c0, c1 in zip(sp[:-1], sp[1:]):
                    nc.scalar.activation(
                        out=stg[0:P, cpar, c0:c1],
                        in_=taub[0:P, c0:c1],
                        func=AF.Exp,
                        bias=c_[0:P, j : j + 1],
                        scale=a_[0:P, j : j + 1],
                    )
                    if len(sp) > 2:
                        nc.sync.dma_start(
                            out=dst[:, cpar, c0:c1], in_=stg[0:P, cpar, c0:c1]
                        )
            if len(sp) == 2:
                nc.sync.dma_start(out=dst, in_=stg[0:P, :, :])

    nc.compile()
    return nc


def _get_nc():
    if "nc" not in _CACHE:
        _CACHE["nc"] = _build_nc()
    return _CACHE["nc"]


def _make_in_maps(inputs_arr, W, b):
    wk = np.ascontiguousarray(inputs_arr[:WORKERS, :AB], dtype=np.float32)
    tau_flat = np.ascontiguousarray(
        inputs_arr[WORKERS:, :ET], dtype=np.float32
    ).reshape(F)
    W = np.ascontiguousarray(W, dtype=np.float32)
    b = np.ascontiguousarray(b, dtype=np.float32)
    ones = np.ones((1, 128), dtype=np.float16)
    maps = []
    for c in range(NCORES):
        tf16 = np.ascontiguousarray(
            tau_flat[c * FS : (c + 1) * FS].astype(np.float16)
        ).reshape(1, FS)
        maps.append({"wk": wk, "tf16": tf16, "ones": ones, "W": W, "b": b})
    return maps


def _run(inputs_arr, W, b, **kwargs):
    from concourse import bass_utils

    nc = _get_nc()
    in_maps = _make_in_maps(inputs_arr, W, b)
    return bass_utils.run_bass_kernel_spmd(
        nc, in_maps, core_ids=list(range(NCORES)), **kwargs
    )


def kernel(inputs, W, b):
    inputs_arr = np.asarray(inputs, dtype=np.float32)
    last_err = None
    for _ in range(3):  # retry transient device failures
        try:
            res = _run(inputs_arr, np.asarray(W), np.asarray(b))
            break
        except Exception as e:  # noqa: BLE001
            last_err = e
    else:
        raise last_err
    out = np.concatenate([r["out"] for r in res.results], axis=1)
    return out.reshape(WORKERS, TASKS, ET)


# revision 14
# speedup vs baseline: 1.0311x; 1.0023x over previous
"""Trainium2 Bass kernel for nn_Decoder_36636071035490.

Reference computes, for workers i and task/edge (j,l):
    z = worker_feature @ W            # [2000, 1]
    p1 = sigmoid(z + b)
    p2 = (1 - p1) / 9
    P[i, j, l] = p1_i^tau_jl * p2_i^(1 - tau_jl)      # [2000, 5000, 10] f32

Identities used on device (exact in exact arithmetic):
    a_i  = (z_i + b) + ln 9           # ln(p1/p2), since logit(sigmoid(x)) = x
    p2_i = 1 / (9 * (1 + exp(z_i + b)))
    c_i  = ln(p2_i) = -ln(1 + exp(z_i + b)) - ln 9
    P[i, f] = exp(a_i * tau_f + c_i)  = p2_i * exp(a_i * tau_f)

Sharding: by output columns (task*edge flattened, 50000 -> 8 x 6250); every
core computes the cheap per-worker scalars for all 2000 workers (replicated
matvec) and produces the full-height [2000, 6250] slab of P.

Device-side schedule (tuned against SDMA-engine traces):
- workers are processed in blocks interleaved two-per-partition (partition
  p of a block holds workers off+2p and off+2p+1), so a block's store is
  one DMA whose per-partition descriptor is a CONTIGUOUS 50 KB range of the
  output (near the 64 KB cap).  128-partition stores get the port-aligned
  16-engine split and run ~26.7 GB/s per SDMA engine; any other partition
  count falls back to a consecutive split at roughly HALF that rate on
  (largest divisor <= 16) engines - measured.
- SDMA engine 15 also serves the dynamic-queue rings and only manages
  ~21 GB/s, so it must carry fewer bytes than engines 0-14.  The only
  full-rate stores are 128-partition ones (which load all 16 engines
  evenly), so the skew comes from ONE half-rate block: 7 blocks x 256
  workers (128 partitions, all 16 engines) + 1 block x 208 workers (104
  partitions -> engines 0..12 only, engine 15 idle).  Exact cover of 2000
  workers, no double stores.
- tau is passed as fp16 [1, 6250] (worst-case P error ~3e-3, budget 2e-2)
  and broadcast across partitions entirely by the idle PE: single-pass
  fp16 matmuls ones[1,128]^T @ tau-chunk -> PSUM bank, DVE copies out.
  No replicated HBM read at all; the 13-chunk chain finishes by ~17 us.
- the per-worker prologue is SPLIT: a mini-prologue for block 0 only
  (workers 0..255, loaded by a dedicated first-in-queue DMA) unblocks the
  first ACT before the PSUM-copy chain occupies the DVE; the rest of the
  prologue runs later.  Block 0 ramps with scale-only Exp quarters
  multiplied by p2 on the DVE, each quarter stored immediately.  The Ln
  chain producing the bias c runs after the quarters; blocks 1+ run pure
  exp(a*tau+c) ACTs with NO steady-state DVE traffic (a per-block DVE
  post-multiply measurably degrades concurrent SDMA store throughput).
"""

import numpy as np

WORKERS = 2000
TASKS = 5000
ET = 10
AB = 64
NCORES = 8
F = TASKS * ET  # 50000 output cols
FS = F // NCORES  # 6250 cols per core
LN9 = float(np.log(9.0))

# 7 blocks of 256 workers (128 partitions, full-rate stores) + 1 block of
# 208 workers (104 partitions -> engines 0..12, skews work off engine 15)
BLOCKS = [(g * 256, 128) for g in range(7)] + [(1792, 104)]
# processing order: ramp block, one more 128p block, then the slow block
# early (its packets sit in the per-engine FIFOs; order doesn't change
# engine finish times but keeps the kernel tail all full-rate)
ORDER = [0, 1, 7, 2, 3, 4, 5, 6]
_Q = [0, 1536, 3072, 4608, FS]  # block-0 quarters, PE-chunk aligned
_H = [0, FS // 2, FS]  # block-1 halves

_CACHE = {}


def _build_nc():
    import concourse.bass as bass
    import concourse.mybir as mybir
    from concourse import bacc
    from concourse.tile import TileContext
    from contextlib import ExitStack

    f32 = mybir.dt.float32
    f16 = mybir.dt.float16
    AF = mybir.ActivationFunctionType
    OP = mybir.AluOpType

    nc = bacc.Bacc("TRN2")
    wk = nc.dram_tensor("wk", [WORKERS, AB], f32, kind="ExternalInput")
    tf16 = nc.dram_tensor("tf16", [1, FS], f16, kind="ExternalInput")
    onesd = nc.dram_tensor("ones", [1, 128], f16, kind="ExternalInput")
    Wd = nc.dram_tensor("W", [AB, 1], f32, kind="ExternalInput")
    bd = nc.dram_tensor("b", [1], f32, kind="ExternalInput")
    out = nc.dram_tensor("out", [WORKERS, FS], f32, kind="ExternalOutput")

    with TileContext(nc) as tc, ExitStack() as ctx:
        const = ctx.enter_context(tc.tile_pool(name="const", bufs=1))
        psum = ctx.enter_context(
            tc.tile_pool(name="ps", bufs=4, space=bass.MemorySpace.PSUM)
        )
        stage_p = ctx.enter_context(tc.tile_pool(name="stagep", bufs=3))

        taub = const.tile([128, FS], f32, name="taub")
        t16 = const.tile([1, FS], f16, name="t16")
        ones_t = const.tile([1, 128], f16, name="ones")
        wka = const.tile([128, 8, 2 * AB], f32, name="wka")
        Wb = const.tile([128, AB], f32, name="Wb")
        bcol = const.tile([128, 1], f32, name="bcol")

        # ---- input DMAs (sync queue is FIFO).  Everything the
        # mini-prologue and PE chain need goes first, in dependency order:
        # block-0 workers, W/b (gate the matvec AND ScalarE's first
        # table-load wait), then fp16 tau + ones for the PE.
        nc.sync.dma_start(
            out=wka[:, 0:1, :],
            in_=wk[0:256, :].rearrange("(g p c) a -> p g (c a)", p=128, c=2),
        )
        nc.sync.dma_start(
            out=Wb, in_=Wd[:].rearrange("a b -> b a").to_broadcast((128, AB))
        )
        nc.sync.dma_start(out=bcol, in_=bd[:].to_broadcast((128, 1)))
        nc.sync.dma_start(out=t16, in_=tf16[:])
        nc.sync.dma_start(out=ones_t, in_=onesd[:])
        # remaining worker features: wka[p, g, (c a)] = wk[off_g + 2p + c, a]
        nc.sync.dma_start(
            out=wka[:, 1:7, :],
            in_=wk[256:1792, :].rearrange("(g p c) a -> p g (c a)", p=128, c=2),
        )
        nc.sync.dma_start(
            out=wka[0:104, 7:8, :],
            in_=wk[1792:2000, :].rearrange("(g p c) a -> p g (c a)", p=104, c=2),
        )
        # pad unused lanes of the 104-partition block with valid data (any
        # rows) so the prologue never reads uninitialized SBUF
        nc.sync.dma_start(
            out=wka[104:128, 7:8, :],
            in_=wk[0:48, :].rearrange("(g p c) a -> p g (c a)", p=24, c=2),
        )

        # ---- mini-prologue: a and p2 for block 0 only (columns j=0,1).
        NJ = 16
        wk16 = wka[:].rearrange("p g (c a) -> p (g c) a", a=AB)
        WbT2 = bass.AP(
            tensor=Wb.tensor,
            offset=Wb.offset,
            ap=[list(Wb.ap[0]), [0, 2], [1, AB]],
        )
        prod = const.tile([128, NJ, AB], f32, name="prod")
        zb_ = const.tile([128, NJ], f32, name="zb")
        a_ = const.tile([128, NJ], f32, name="a")
        eb_ = const.tile([128, NJ], f32, name="eb")
        nc.vector.tensor_mul(prod[:, 0:2, :], wk16[:, 0:2, :], WbT2)
        nc.vector.reduce_sum(
            out=zb_[:, 0:2].rearrange("p (t o) -> p t o", o=1),
            in_=prod[:, 0:2, :],
            axis=mybir.AxisListType.X,
        )
        nc.vector.tensor_scalar(
            out=a_[:, 0:2],
            in0=zb_[:, 0:2],
            scalar1=bcol,
            scalar2=LN9,
            op0=OP.add,
            op1=OP.add,
        )
        nc.scalar.activation(
            out=eb_[:, 0:2], in_=zb_[:, 0:2], func=AF.Exp, bias=bcol, scale=1.0
        )
        den_ = const.tile([128, 2], f32, name="den")
        nc.vector.tensor_scalar(
            out=den_, in0=eb_[:, 0:2], scalar1=1.0, scalar2=9.0, op0=OP.add, op1=OP.mult
        )
        p2_ = const.tile([128, 2], f32, name="p2")
        nc.vector.reciprocal(out=p2_, in_=den_)

        # ---- PE partition-broadcast of tau (fp16 single-pass matmuls)
        CH = 512  # one PSUM bank of f32
        chunks = []
        for n0 in range(0, FS, CH):
            n1 = min(n0 + CH, FS)
            ps = psum.tile([128, CH], f32, name="ps", tag="ps")
            nc.tensor.matmul(
                ps[:, 0 : n1 - n0],
                ones_t[:],
                t16[0:1, n0:n1],
                start=True,
                stop=True,
            )
            chunks.append((n0, n1, ps))
        for n0, n1, ps in chunks:
            nc.vector.tensor_copy(taub[:, n0:n1], ps[:, 0 : n1 - n0])

        # ---- main prologue: z, a for all 16 columns (j = 2g + c_par holds
        # worker off_g + 2p + c_par)
        WbT = bass.AP(
            tensor=Wb.tensor,
            offset=Wb.offset,
            ap=[list(Wb.ap[0]), [0, NJ - 2], [1, AB]],
        )
        nc.vector.tensor_mul(prod[:, 2:NJ, :], wk16[:, 2:NJ, :], WbT)
        nc.vector.reduce_sum(
            out=zb_[:, 2:NJ].rearrange("p (t o) -> p t o", o=1),
            in_=prod[:, 2:NJ, :],
            axis=mybir.AxisListType.X,
        )
        nc.vector.tensor_scalar(
            out=a_[:, 2:NJ],
            in0=zb_[:, 2:NJ],
            scalar1=bcol,
            scalar2=LN9,
            op0=OP.add,
            op1=OP.add,
        )

        # ---- block 0 ramp: scale-only Exp quarters, DVE multiplies by p2
        stg0 = stage_p.tile([128, 2, FS], f32, name="stg0", tag="stg")
        dst0 = out[0:256, :].rearrange("(p c) f -> p c f", c=2)
        for qi in range(4):
            c0, c1 = _Q[qi], _Q[qi + 1]
            for cpar in (0, 1):
                nc.scalar.activation(
                    out=stg0[:, cpar, c0:c1],
                    in_=taub[:, c0:c1],
                    func=AF.Exp,
                    scale=a_[:, cpar : cpar + 1],
                )
                nc.vector.tensor_scalar_mul(
                    stg0[:, cpar, c0:c1],
                    stg0[:, cpar, c0:c1],
                    p2_[:, cpar : cpar + 1],
                )
                nc.sync.dma_start(
                    out=dst0[:, cpar, c0:c1], in_=stg0[:, cpar, c0:c1]
                )

        # ---- the bias c for blocks 1+: c = -ln(1 + e^(z+b)) - ln 9
        lb_ = const.tile([128, NJ], f32, name="lb")
        c_ = const.tile([128, NJ], f32, name="c")
        nc.scalar.activation(
            out=eb_[:, 2:NJ], in_=zb_[:, 2:NJ], func=AF.Exp, bias=bcol, scale=1.0
        )
        nc.scalar.activation(out=lb_, in_=eb_, func=AF.Ln, bias=1.0, scale=1.0)
        nc.vector.tensor_scalar(
            out=c_, in0=lb_, scalar1=-1.0, scalar2=-LN9, op0=OP.mult, op1=OP.add
        )

        # ---- remaining blocks: pure exp(a*tau + c) ACTs, no DVE traffic
        for g in ORDER[1:]:
            off, P = BLOCKS[g]
            stg = stage_p.tile([128, 2, FS], f32, name="stg", tag="stg")
            dst = out[off : off + 2 * P, :].rearrange("(p c) f -> p c f", c=2)
            sp = _H if g == 1 else [0, FS]
            for cpar in (0, 1):
                j = 2 * g + cpar
                for c0, c1 in zip(sp[:-1], sp[1:]):
                    nc.scalar.activation(
                        out=stg[0:P, cpar, c0:c1],
                        in_=taub[0:P, c0:c1],
                        func=AF.Exp,
                        bias=c_[0:P, j : j + 1],
                        scale=a_[0:P, j : j + 1],
                    )
                    if len(sp) > 2:
                        nc.sync.dma_start(
                            out=dst[:, cpar, c0:c1], in_=stg[0:P, cpar, c0:c1]
                        )
            if len(sp) == 2:
                nc.sync.dma_start(out=dst, in_=stg[0:P, :, :])

    nc.compile()
    return nc


def _get_nc():
    if "nc" not in _CACHE:
        _CACHE["nc"] = _build_nc()
    return _CACHE["nc"]


def _make_in_maps(inputs_arr, W, b):
    wk = np.ascontiguousarray(inputs_arr[:WORKERS, :AB], dtype=np.float32)
    tau_flat = np.ascontiguousarray(
        inputs_arr[WORKERS:, :ET], dtype=np.float32
    ).reshape(F)
    W = np.ascontiguousarray(W, dtype=np.float32)
    b = np.ascontiguousarray(b, dtype=np.float32)
    ones = np.ones((1, 128), dtype=np.float16)
    maps = []
    for c in range(NCORES):
        tf16 = np.ascontiguousarray(
            tau_flat[c * FS : (c + 1) * FS].astype(np.float16)
        ).reshape(1, FS)
        maps.append({"wk": wk, "tf16": tf16, "ones": ones, "W": W, "b": b})
    return maps


def _run(inputs_arr, W, b, **kwargs):
    from concourse import bass_utils

    nc = _get_nc()
    in_maps = _make_in_maps(inputs_arr, W, b)
    return bass_utils.run_bass_kernel_spmd(
        nc, in_maps, core_ids=list(range(NCORES)), **kwargs
    )


def kernel(inputs, W, b):
    inputs_arr = np.asarray(inputs, dtype=np.float32)
    last_err = None
    for _ in range(3):  # retry transient device failures
        try:
            res = _run(inputs_arr, np.asarray(W), np.asarray(b))
            break
        except Exception as e:  # noqa: BLE001
            last_err = e
    else:
        raise last_err
    out = np.concatenate([r["out"] for r in res.results], axis=1)
    return out.reshape(WORKERS, TASKS, ET)
